# revision 1
# baseline (speedup 1.0000x reference)
"""Trainium2 Bass kernel for nn_HadamardModule (SORF random-feature module).

Reference computation:
    x_ = x @ projector                      # [N=8192, 128]
    y = broadcast over 64 stacks
    for t in 0,1: y = COEFF * fwht(d[t] * y)
    out = cos(y.reshape(N, 8192) + 2*pi*b)

Key identity: fwht over 128 elems == multiply by symmetric Hadamard matrix H.
The whole per-stack SORF transform is linear:
    feats[:, s] = x_ @ A_s,   A_s = COEFF^2 * diag(d0_s) @ H @ diag(d1_s) @ H
A_s/2pi is folded ON DEVICE (H @ (d1_s * H) is an exact integer matmul, then a
per-partition scale by COEFF^2/(2pi) * d0_s); the main loop computes:
    z0 = (x @ P) @ (A_s / 2pi)            # phase in periods, via TensorE fp32
    r  = z0' - round(z0')                 # range reduction, z0' = z0 + frac-bias
    out = sin(2*pi*r)                     # ScalarE Sin LUT (valid on [-pi, pi])
round() uses the fp32 magic-number trick ((v + 1.5*2^23) - 1.5*2^23), in
2 VectorE passes; the per-feature bias rides in the first pass and in the
Sin activation's per-partition bias so everything stays exact:
    tmid = (z0 + b'') + M                  # = round(z0 + b'') + M, exact
    u    = (tmid - M) - z0                 # = b'' - r, exact (cancellation)
    out  = Sin(-2pi * u + 2pi * b'')       # = sin(2pi * r),  arg in [-pi, pi]
Output is written bf16 (quantization ~1e-3, well under the ~9e-3 fp32 noise
floor of this phase-sensitive computation) and upcast on the host.

Sharding: data-parallel over the 8192 rows -> 1024 rows per core on 8 cores.
x is passed pre-transposed (features on partitions) so no device transposes
are needed; the output comes back feature-major per 128-feature stack block
and is re-assembled on the host.
"""

import concurrent.futures as _futures

import numpy as np

NPCAS = 128
OUT_DIM = 8192
NSTACKS = 64
COEFF = np.sqrt(np.float64(NPCAS)) / 3.0
TWO_PI = 2.0 * np.pi
C_SCALE = float(COEFF**2 / TWO_PI)
N_CORES = 8
ROWS = 8192
ROWS_PER_CORE = ROWS // N_CORES  # 1024
CHUNK = 512
N_CHUNKS = ROWS_PER_CORE // CHUNK  # 2
MAGIC = float(np.float32(1.5 * 2**23))

_cached = {}


def _hadamard128():
    H = np.array([[1.0]])
    while H.shape[0] < NPCAS:
        H = np.block([[H, H], [H, -H]])
    return H


# fraction of stacks whose bias-add pass runs on ScalarE (rest on VectorE);
# tuned so ACT (bias + Sin) and DVE (bias-remainder + subtract) balance.
ACT_PASS1_MOD = (5, 8)  # s % 8 < 5 -> ACT


def _build_nc():
    import concourse.bacc as bacc
    import concourse.mybir as mybir
    import concourse.tile as tile

    f32 = mybir.dt.float32
    bf16 = mybir.dt.bfloat16
    add = mybir.AluOpType.add
    sub = mybir.AluOpType.subtract
    mult = mybir.AluOpType.mult

    nc = bacc.Bacc("TRN2", target_bir_lowering=False, debug=False)
    xT = nc.dram_tensor("xT", [4, 128, ROWS_PER_CORE], f32, kind="ExternalInput")
    Pc = nc.dram_tensor("Pc", [4, 128, 128], f32, kind="ExternalInput")
    Hd = nc.dram_tensor("Hd", [128, 128], f32, kind="ExternalInput")
    d0d = nc.dram_tensor("d0d", [128, NSTACKS], f32, kind="ExternalInput")
    d1d = nc.dram_tensor("d1d", [128, NSTACKS], f32, kind="ExternalInput")
    b1d = nc.dram_tensor("b1d", [128, NSTACKS], f32, kind="ExternalInput")
    out = nc.dram_tensor(
        "out", [NSTACKS, 128, ROWS_PER_CORE], bf16, kind="ExternalOutput"
    )

    with tile.TileContext(nc) as tc:
        with (
            tc.tile_pool(name="const", bufs=1) as const,
            tc.tile_pool(name="psum_fp", bufs=2, space="PSUM") as psum_fp,
            tc.tile_pool(name="psum_z", bufs=3, space="PSUM") as psum_z,
            tc.tile_pool(name="fold", bufs=2) as foldp,
            tc.tile_pool(name="work", bufs=3) as work,
            tc.tile_pool(name="outp", bufs=4) as outp,
        ):
            Pt = const.tile([128, 4, 128], f32)
            Xt = const.tile([128, 4, ROWS_PER_CORE], f32)
            for k in range(4):
                nc.sync.dma_start(Pt[:, k, :], Pc[k])
                nc.sync.dma_start(Xt[:, k, :], xT[k])
            Ht = const.tile([128, 128], f32)
            nc.sync.dma_start(Ht[:], Hd[:])
            d0t = const.tile([128, NSTACKS], f32)
            d1t = const.tile([128, NSTACKS], f32)
            nc.sync.dma_start(d0t[:], d0d[:])
            nc.sync.dma_start(d1t[:], d1d[:])
            b1 = const.tile([128, NSTACKS], f32)
            nc.sync.dma_start(b1[:], b1d[:])

            # fold A_s/2pi = (C*d0_s) * (H @ (d1_s * H)) on device.
            # H @ (d1*H) is exact (integer entries <= 128 in fp32 accum);
            # d0t is pre-scaled by C_SCALE/2pi on the host.
            At = const.tile([128, NSTACKS, 128], f32)
            for s in range(NSTACKS):
                w1 = foldp.tile([128, 128], f32, tag="w1")
                nc.vector.tensor_scalar(w1[:], Ht[:], d1t[:, s : s + 1], None, mult)
                pin = psum_fp.tile([128, CHUNK], f32, tag="fp")
                nc.tensor.matmul(
                    pin[:, :128], Ht[:], w1[:], start=True, stop=True
                )
                nc.scalar.mul(At[:, s, :], pin[:, :128], d0t[:, s : s + 1])

            # projection: x_^T = P^T @ x^T, K=512 in 4 chunks of 128
            xsb = const.tile([128, N_CHUNKS, CHUNK], f32)
            for c in range(N_CHUNKS):
                pp = psum_fp.tile([128, CHUNK], f32, tag="fp")
                for k in range(4):
                    nc.tensor.matmul(
                        pp[:],
                        Pt[:, k, :],
                        Xt[:, k, c * CHUNK : (c + 1) * CHUNK],
                        start=(k == 0),
                        stop=(k == 3),
                    )
                nc.vector.tensor_copy(xsb[:, c, :], pp[:])

            # per-stack (FD=1024 spans both row chunks / 2 PSUM banks):
            #   z0 = x_ @ A_s/2pi; w = z0 + b''; t2 = round(w); r = w - t2;
            #   out = sin(2pi*r)
            for s in range(NSTACKS):
                z0 = psum_z.tile([128, ROWS_PER_CORE], f32)
                for c in range(N_CHUNKS):
                    nc.tensor.matmul(
                        z0[:, c * CHUNK : (c + 1) * CHUNK],
                        At[:, s, :],
                        xsb[:, c, :],
                        start=True,
                        stop=True,
                    )
                w = work.tile([128, ROWS_PER_CORE], f32, tag="w")
                if s % ACT_PASS1_MOD[1] < ACT_PASS1_MOD[0]:
                    nc.scalar.activation(
                        w[:],
                        z0[:],
                        mybir.ActivationFunctionType.Identity,
                        bias=b1[:, s : s + 1],
                        scale=1.0,
                    )
                else:
                    nc.vector.tensor_scalar(
                        w[:], z0[:], b1[:, s : s + 1], None, add
                    )
                t2 = work.tile([128, ROWS_PER_CORE], f32, tag="t2")
                nc.gpsimd.tensor_scalar(t2[:], w[:], MAGIC, MAGIC, add, sub)
                r = work.tile([128, ROWS_PER_CORE], f32, tag="r")
                nc.vector.tensor_tensor(r[:], w[:], t2[:], sub)
                osb = outp.tile([128, ROWS_PER_CORE], bf16)
                nc.scalar.activation(
                    osb[:],
                    r[:],
                    mybir.ActivationFunctionType.Sin,
                    bias=0.0,
                    scale=TWO_PI,
                )
                nc.sync.dma_start(out[s], osb[:])

    nc.compile()
    return nc


def _make_runner():
    """Compile once and build a persistent jitted SPMD executable.

    Adapted from concourse.bass2jax.run_bass_via_pjrt, but cached across
    calls: x shards across the 8 cores, the small operands broadcast, and
    the zero output buffers live on device (not donated, reused each call;
    the NEFF overwrites every element of `out`).
    """
    import jax
    import concourse.mybir as mybir
    from jax.experimental.shard_map import shard_map
    from jax.sharding import Mesh, NamedSharding, PartitionSpec
    from concourse.bass2jax import (
        _bass_exec_p,
        install_neuronx_cc_hook,
        partition_id_tensor,
    )

    nc = _build_nc()
    _cached["nc"] = nc
    install_neuronx_cc_hook()

    partition_name = (
        nc.partition_id_tensor.name if nc.partition_id_tensor else None
    )
    in_names, out_names, out_avals = [], [], []
    for alloc in nc.m.functions[0].allocations:
        if not isinstance(alloc, mybir.MemoryLocationSet):
            continue
        name = alloc.memorylocations[0].name
        if alloc.kind == "ExternalInput":
            if name != partition_name:
                in_names.append(name)
        elif alloc.kind == "ExternalOutput":
            out_names.append(name)
            out_avals.append(
                jax.core.ShapedArray(
                    tuple(alloc.tensor_shape), mybir.dt.np(alloc.dtype)
                )
            )

    sharded_inputs = {"xT"}
    call_names = tuple(in_names) + tuple(out_names)
    if partition_name is not None:
        call_names = call_names + (partition_name,)

    def _body(*args):
        extra = [partition_id_tensor()] if partition_name is not None else []
        outs = _bass_exec_p.bind(
            *args,
            *extra,
            out_avals=tuple(out_avals),
            in_names=call_names,
            out_names=tuple(out_names),
            lowering_input_output_aliases=(),
            sim_require_finite=True,
            sim_require_nnan=True,
            nc=nc,
        )
        return tuple(outs)

    devices = jax.devices()[:N_CORES]
    mesh = Mesh(np.asarray(devices), ("core",))
    in_specs = tuple(
        PartitionSpec("core") if n in sharded_inputs else PartitionSpec()
        for n in in_names
    ) + (PartitionSpec("core"),) * len(out_names)
    out_specs = (PartitionSpec("core"),) * len(out_names)
    fn = jax.jit(
        shard_map(
            _body, mesh=mesh, in_specs=in_specs, out_specs=out_specs, check_rep=False
        )
    )

    # device-resident zero output buffers, transferred once and reused
    zeros = [
        jax.device_put(
            np.zeros((N_CORES * a.shape[0], *a.shape[1:]), a.dtype),
            NamedSharding(mesh, PartitionSpec("core")),
        )
        for a in out_avals
    ]
    return fn, in_names, zeros


def _get_runner():
    if "runner" not in _cached:
        _cached["runner"] = _make_runner()
    return _cached["runner"]


def _host_prep(x, projector, d, b):
    """Build device inputs; the SORF fold itself happens on device."""
    H = np.ascontiguousarray(_hadamard128(), dtype=np.float32)
    d32 = d.astype(np.float32)
    d0 = np.ascontiguousarray(d32[0].T * np.float32(C_SCALE))  # [128, 64], scaled
    d1 = np.ascontiguousarray(d32[1].T)  # [128, 64]

    # phase bias in periods: b' = b + 0.25 (cos -> sin); b'' = b' - round(b')
    bp = b.astype(np.float64) + 0.25
    bpp = bp - np.round(bp)  # in [-0.5, 0.5]
    b1 = np.ascontiguousarray(bpp.reshape(NSTACKS, 128).T.astype(np.float32))

    Pc = np.ascontiguousarray(projector.astype(np.float32).reshape(4, 128, 128))

    # global xT: [8*4, 128, 1024]; shard_map slices axis 0 per core
    x2 = x.astype(np.float32).reshape(ROWS, 512)
    xT = np.empty((N_CORES, 4, 128, ROWS_PER_CORE), np.float32)
    for core in range(N_CORES):
        xs = x2[core * ROWS_PER_CORE : (core + 1) * ROWS_PER_CORE]
        xT[core] = xs.T.reshape(4, 128, ROWS_PER_CORE)
    xT = xT.reshape(N_CORES * 4, 128, ROWS_PER_CORE)
    return {
        "xT": xT, "Pc": Pc, "Hd": H, "d0d": d0, "d1d": d1, "b1d": b1
    }


def _assemble(out_global):
    """core-sharded [8*64, 128, 1024] bf16 -> [64, 128, 8192] fp32."""
    full = np.empty((ROWS, OUT_DIM), np.float32)
    view = full.reshape(N_CORES, ROWS_PER_CORE, NSTACKS, 128)

    shards = sorted(
        out_global.addressable_shards, key=lambda s: s.index[0].start or 0
    )

    def fetch(i):
        o = np.asarray(shards[i].data)  # [64, 128, 1024] bf16
        # o[s, m, j] -> view[i, j, s, m], upcast bf16 -> fp32
        np.copyto(view[i], o.transpose(2, 0, 1))

    with _futures.ThreadPoolExecutor(max_workers=N_CORES) as ex:
        list(ex.map(fetch, range(N_CORES)))
    return full.reshape(64, 128, OUT_DIM)


def kernel(x, projector, d, b):
    fn, in_names, zeros = _get_runner()
    ins = _host_prep(
        np.asarray(x), np.asarray(projector), np.asarray(d), np.asarray(b)
    )
    outs = fn(*[ins[n] for n in in_names], *zeros)
    return _assemble(outs[0])

